# revision 4
# baseline (speedup 1.0000x reference)
"""Trainium2 Bass kernel for the CustomRNN problem.

Math (per batch row):
    h_t   = tanh(x_t @ W1 + b1)                 (parallel over t)
    y_t   = h_t + tanh(y_{t-1} @ W2 + b2)       (serial scan over t)
    out_t = y_t @ Wc + bc                       (parallel over t)

Strategy (8 cores, data-parallel over batch; B_LOC = 32 rows/core):
  * Everything on-chip is kept in "transposed" layout [U, n] with column
    index n = b*T + t (b-major), so the scan's matmul keeps W2 as the
    stationary PE operand across all 512 steps.
  * Scan recurrence is rewritten with s_t := y_{t-1} @ W2 + b2-less form:
        g_t   = h_t @ W2                (big parallel GEMM, accumulated
                                         directly into the scan PSUM banks)
        tau_t = tanh(s_t + b2)          (ACT, reads PSUM, writes SBUF)
        s_{t+1} = g_t + tau_t @ W2      (PE matmul accumulate, start=False)
        y_t   = h_t + tau_t             (DVE, off the critical path)
    so the serial critical path is exactly PE -> ACT -> PE per step.
  * x is transposed on-chip via bf16 DMA-xbar transposes (no PE cost).
  * PSUM banks hold 16 scan slots of [128, 32] each, in (t, b) order via
    strided access patterns; SBUF buffers stay b-major.
"""

import numpy as np

import concourse.bacc as bacc
import concourse.bass as bass
import concourse.mybir as mybir
import concourse.tile as tile
from concourse import bass_utils

B, T, D, U, C = 256, 512, 128, 128, 64
NCORES = 8
BL = B // NCORES  # 32 batch rows per core
P = 128
SLOTS = 16  # scan slots per PSUM bank
NBANKS = T // SLOTS  # 32

f32 = mybir.dt.float32
bf16 = mybir.dt.bfloat16
Tanh = mybir.ActivationFunctionType.Tanh


def build_body(nc, tc, ctx, x, w1d, b1d, w2d, b2d, wcd, bcd, outd):
    const = ctx.enter_context(tc.tile_pool(name="const", bufs=1))
    big = ctx.enter_context(tc.tile_pool(name="big", bufs=1))

    # ---- constants ----
    w1f = const.tile([D, U], f32)
    nc.sync.dma_start(w1f[:], w1d[:])
    w1s = const.tile([D, U], bf16)
    nc.vector.tensor_copy(w1s[:], w1f[:])
    w2s = const.tile([U, U], f32)
    nc.sync.dma_start(w2s[:], w2d[:])
    wcf = const.tile([U, C], f32)
    nc.sync.dma_start(wcf[:], wcd[:])
    wcb = const.tile([U, C], bf16)
    nc.vector.tensor_copy(wcb[:], wcf[:])
    b1s = const.tile([U, 1], f32)
    nc.sync.dma_start(b1s[:], b1d.unsqueeze(1))
    b2s = const.tile([U, 1], f32)
    nc.sync.dma_start(b2s[:], b2d.unsqueeze(1))
    zero32 = const.tile([U, BL], f32)
    nc.vector.memset(zero32[:], 0.0)
    ones1 = const.tile([1, P], f32)
    nc.vector.memset(ones1[:], 1.0)
    bc1 = const.tile([1, C], f32)
    nc.sync.dma_start(bc1[:], bcd.unsqueeze(0))

    # ---- big SBUF buffers (column index n = b*T + t) ----
    hbuf = big.tile([P, BL * T], f32)   # h_t, transposed layout
    taub = big.tile([P, BL * T], f32)   # tau_t
    ybuf = big.tile([P, BL * T], bf16)  # y_t = h_t + tau_t

    # strided views: [p, t, b]
    Hv = hbuf[:].rearrange("p (b t) -> p t b", b=BL, t=T)
    Tv = taub[:].rearrange("p (b t) -> p t b", b=BL, t=T)
    Yv = ybuf[:].rearrange("p (b t) -> p t b", b=BL, t=T)

    # ---- phase A: x load, cast, xbar-transpose, input GEMM ----
    xa_pool = ctx.enter_context(tc.tile_pool(name="xa", bufs=3))
    xb_pool = ctx.enter_context(tc.tile_pool(name="xb", bufs=3))
    xt_pool = ctx.enter_context(tc.tile_pool(name="xt", bufs=3))
    ph_psum = ctx.enter_context(tc.tile_pool(name="ph", bufs=2, space="PSUM"))

    # bc broadcast tile via K=1 matmul (bcb4 = ones^T @ bc, tiled 4x)
    psmall = ph_psum.tile([P, C], f32, tag="ph")
    nc.tensor.matmul(psmall[:], lhsT=ones1[:], rhs=bc1[:], start=True, stop=True)
    bcb4 = const.tile([P, 4 * C], f32)
    for k in range(4):
        nc.vector.tensor_copy(bcb4[:, k * C:(k + 1) * C], psmall[:])

    for b in range(BL):
        xa = xa_pool.tile([P, T], f32)
        # x[b] is [T, D]; load rows t = a*128 + p onto partition p
        nc.sync.dma_start(xa[:], x[b].rearrange("(a p) d -> p a d", p=P))
        xb = xb_pool.tile([P, T], bf16)
        nc.vector.tensor_copy(xb[:], xa[:])
        xt = xt_pool.tile([P, T], bf16)
        for a in range(4):
            nc.sync.dma_start_transpose(
                xt[:, a * P:(a + 1) * P], xb[:, a * P:(a + 1) * P]
            )
        ph = ph_psum.tile([P, T], f32, tag="ph")
        nc.tensor.matmul(ph[:], lhsT=w1s[:], rhs=xt[:], start=True, stop=True)
        nc.scalar.activation(hbuf[:, b * T:(b + 1) * T], ph[:], Tanh, bias=b1s[:])

    # ---- phase B: the serial scan ----
    scan_psum = ctx.enter_context(tc.tile_pool(name="scan", bufs=4, space="PSUM"))

    # tau_0 = tanh(0 + b2)
    nc.scalar.activation(Tv[:, 0, :], zero32[:], Tanh, bias=b2s[:])

    bank = None
    for t in range(T):
        m, sl = divmod(t, SLOTS)
        if sl == 0:
            bank = scan_psum.tile([P, SLOTS * BL], f32, tag="bank")
            # g for this bank: slots sl' hold g_{16m+sl'} = h_{16m+sl'} @ W2
            # rhs columns iterate (t', b) to match slot order.
            nc.tensor.matmul(
                bank[:],
                lhsT=w2s[:],
                rhs=Hv[:, m * SLOTS:(m + 1) * SLOTS, :],
                start=True,
                stop=False,
                skip_group_check=True,
            )
        slot = bank[:, sl * BL:(sl + 1) * BL]
        if t < T - 1:
            # s_{t+1} += tau_t @ W2
            nc.tensor.matmul(
                slot,
                lhsT=w2s[:],
                rhs=Tv[:, t, :],
                start=False,
                stop=True,
                skip_group_check=True,
            )
            # tau_{t+1} = tanh(s_{t+1} + b2)
            nc.scalar.activation(Tv[:, t + 1, :], slot, Tanh, bias=b2s[:])
        if sl == SLOTS - 1:
            # y = h + tau for this bank's t-range (off critical path, DVE)
            nc.vector.tensor_add(
                Yv[:, m * SLOTS:(m + 1) * SLOTS, :],
                Hv[:, m * SLOTS:(m + 1) * SLOTS, :],
                Tv[:, m * SLOTS:(m + 1) * SLOTS, :],
            )

    # ---- phase C: classifier out = y @ Wc + bc ----
    cls_psum = ctx.enter_context(tc.tile_pool(name="cls", bufs=2, space="PSUM"))
    osb_pool = ctx.enter_context(tc.tile_pool(name="osb", bufs=3))
    for b in range(BL):
        ps = cls_psum.tile([P, 4 * C], f32, tag="cls")
        for k in range(4):
            nc.tensor.matmul(
                ps[:, k * C:(k + 1) * C],
                lhsT=ybuf[:, b * T + k * P: b * T + (k + 1) * P],
                rhs=wcb[:],
                start=True,
                stop=True,
            )
        osb = osb_pool.tile([P, 4 * C], f32)
        nc.vector.tensor_add(osb[:], ps[:], bcb4[:])
        nc.sync.dma_start(
            outd[b].rearrange("(k p) c -> p k c", p=P),
            osb[:].rearrange("p (k c) -> p k c", c=C),
        )


def build_nc():
    nc = bacc.Bacc("TRN2", target_bir_lowering=False, debug=False,
                   num_devices=NCORES)
    x = nc.dram_tensor("inputs", [BL, T, D], f32, kind="ExternalInput").ap()
    w1 = nc.dram_tensor("W1", [D, U], f32, kind="ExternalInput").ap()
    b1 = nc.dram_tensor("b1", [U], f32, kind="ExternalInput").ap()
    w2 = nc.dram_tensor("W2", [U, U], f32, kind="ExternalInput").ap()
    b2 = nc.dram_tensor("b2", [U], f32, kind="ExternalInput").ap()
    wc = nc.dram_tensor("Wc", [U, C], f32, kind="ExternalInput").ap()
    bc = nc.dram_tensor("bc", [C], f32, kind="ExternalInput").ap()
    out = nc.dram_tensor("out", [BL, T, C], f32, kind="ExternalOutput").ap()

    with tile.TileContext(nc) as tc:
        import contextlib
        with contextlib.ExitStack() as ctx:
            build_body(nc, tc, ctx, x, w1, b1, w2, b2, wc, bc, out)
    nc.finalize()
    return nc


def make_in_maps(inputs):
    xs = np.ascontiguousarray(np.asarray(inputs["inputs"], dtype=np.float32))
    shards = np.split(xs, NCORES, axis=0)
    common = {
        k: np.ascontiguousarray(np.asarray(inputs[k], dtype=np.float32))
        for k in ("W1", "b1", "W2", "b2", "Wc", "bc")
    }
    return [dict(inputs=shards[i], **common) for i in range(NCORES)]


def kernel(**inputs):
    nc = build_nc()
    in_maps = make_in_maps(inputs)
    res = bass_utils.run_bass_kernel_spmd(nc, in_maps, list(range(NCORES)))
    outs = [np.asarray(res.results[i]["out"]) for i in range(NCORES)]
    return np.concatenate(outs, axis=0).astype(np.float32)


# revision 9
# speedup vs baseline: 11.3075x; 11.3075x over previous
"""Trainium2 Bass kernel for the CustomRNN problem.

Math (per batch row):
    h_t   = tanh(x_t @ W1 + b1)                 (parallel over t)
    y_t   = h_t + tanh(y_{t-1} @ W2 + b2)       (serial scan over t)
    out_t = y_t @ Wc + bc                       (parallel over t)

Strategy (8 cores, data-parallel over batch; B_LOC = 32 rows/core):
  * Everything on-chip is kept in "transposed" layout [U, n] with column
    index n = b*T + t (b-major), so the scan's matmul keeps W2 as the
    stationary PE operand across all 512 steps.
  * Scan recurrence is rewritten with s_t := y_{t-1} @ W2 + b2-less form:
        g_t   = h_t @ W2                (big parallel GEMM, accumulated
                                         directly into the scan PSUM banks)
        tau_t = tanh(s_t + b2)          (ACT, reads PSUM, writes SBUF)
        s_{t+1} = g_t + tau_t @ W2      (PE matmul accumulate, start=False)
        y_t   = h_t + tau_t             (DVE, off the critical path)
    so the serial critical path is exactly PE -> ACT -> PE per step.
  * x is transposed on-chip via bf16 DMA-xbar transposes (no PE cost).
  * PSUM banks hold 16 scan slots of [128, 32] each, in (t, b) order via
    strided access patterns; SBUF buffers stay b-major.
"""

import numpy as np

import concourse.bacc as bacc
import concourse.bass as bass
import concourse.mybir as mybir
import concourse.tile as tile
from concourse import bass_utils

B, T, D, U, C = 256, 512, 128, 128, 64
NCORES = 8
BL = B // NCORES  # 32 batch rows per core
P = 128
SLOTS = 16  # scan slots per PSUM bank
NBANKS = T // SLOTS  # 32

f32 = mybir.dt.float32
bf16 = mybir.dt.bfloat16
Tanh = mybir.ActivationFunctionType.Tanh


def build_body(nc, tc, ctx, x, w1d, b1d, w2d, b2d, wcd, bcd, outd, rep=0):
    pfx = f"r{rep}_"
    const = ctx.enter_context(tc.tile_pool(name=pfx + "const", bufs=1))
    big = ctx.enter_context(tc.tile_pool(name=pfx + "big", bufs=1))

    # ---- constants ----
    w1f = const.tile([D, U], f32)
    nc.sync.dma_start(w1f[:], w1d[:])
    w1s = const.tile([D, U], bf16)
    nc.vector.tensor_copy(w1s[:], w1f[:])
    w2s = const.tile([U, U], f32)
    nc.sync.dma_start(w2s[:], w2d[:])
    wcf = const.tile([U, C], f32)
    nc.sync.dma_start(wcf[:], wcd[:])
    wcb = const.tile([U, C], bf16)
    nc.vector.tensor_copy(wcb[:], wcf[:])
    b1s = const.tile([U, 1], f32)
    nc.sync.dma_start(b1s[:], b1d.unsqueeze(1))
    b2s = const.tile([U, 1], f32)
    nc.sync.dma_start(b2s[:], b2d.unsqueeze(1))
    zero32 = const.tile([U, BL], f32)
    nc.vector.memset(zero32[:], 0.0)
    ones1 = const.tile([1, P], f32)
    nc.vector.memset(ones1[:], 1.0)
    bc1 = const.tile([1, C], f32)
    nc.sync.dma_start(bc1[:], bcd.unsqueeze(0))

    # ---- big SBUF buffers (column index n = b*T + t) ----
    hbuf = big.tile([P, BL * T], f32)   # h_t, transposed layout
    taub = big.tile([P, BL * T], f32)   # tau_t
    ybuf = big.tile([P, BL * T], bf16)  # y_t = h_t + tau_t

    # strided views: [p, t, b]
    Hv = hbuf[:].rearrange("p (b t) -> p t b", b=BL, t=T)
    Tv = taub[:].rearrange("p (b t) -> p t b", b=BL, t=T)
    Yv = ybuf[:].rearrange("p (b t) -> p t b", b=BL, t=T)

    # ---- phase A: x load, cast, xbar-transpose, input GEMM ----
    xa_pool = ctx.enter_context(tc.tile_pool(name=pfx + "xa", bufs=3))
    xb_pool = ctx.enter_context(tc.tile_pool(name=pfx + "xb", bufs=3))
    xt_pool = ctx.enter_context(tc.tile_pool(name=pfx + "xt", bufs=3))
    ph_psum = ctx.enter_context(tc.tile_pool(name=pfx + "ph", bufs=2, space="PSUM"))

    # bc broadcast tile via K=1 matmul (bcb4 = ones^T @ bc, tiled 4x)
    psmall = ph_psum.tile([P, C], f32, tag="ph")
    nc.tensor.matmul(psmall[:], lhsT=ones1[:], rhs=bc1[:], start=True, stop=True)
    bcb4 = const.tile([P, 4 * C], f32)
    for k in range(4):
        nc.vector.tensor_copy(bcb4[:, k * C:(k + 1) * C], psmall[:])

    for b in range(BL):
        xa = xa_pool.tile([P, T], f32)
        # x[b] is [T, D]; load rows t = a*128 + p onto partition p
        nc.sync.dma_start(xa[:], x[b].rearrange("(a p) d -> p a d", p=P))
        xb = xb_pool.tile([P, T], bf16)
        nc.vector.tensor_copy(xb[:], xa[:])
        xt = xt_pool.tile([P, T], bf16)
        for a in range(4):
            nc.sync.dma_start_transpose(
                xt[:, a * P:(a + 1) * P], xb[:, a * P:(a + 1) * P]
            )
        ph = ph_psum.tile([P, T], f32, tag="ph")
        nc.tensor.matmul(ph[:], lhsT=w1s[:], rhs=xt[:], start=True, stop=True)
        nc.scalar.activation(hbuf[:, b * T:(b + 1) * T], ph[:], Tanh, bias=b1s[:])

    # ---- phase B: the serial scan ----
    scan_psum = ctx.enter_context(
        tc.tile_pool(name=pfx + "scan", bufs=4, space="PSUM"))

    # tau_0 = tanh(0 + b2)
    nc.scalar.activation(Tv[:, 0, :], zero32[:], Tanh, bias=b2s[:])

    bank = None
    for t in range(T):
        m, sl = divmod(t, SLOTS)
        if sl == 0:
            bank = scan_psum.tile([P, SLOTS * BL], f32, tag="bank")
            # g for this bank: slots sl' hold g_{16m+sl'} = h_{16m+sl'} @ W2
            # rhs columns iterate (t', b) to match slot order.
            nc.tensor.matmul(
                bank[:],
                lhsT=w2s[:],
                rhs=Hv[:, m * SLOTS:(m + 1) * SLOTS, :],
                start=True,
                stop=False,
                skip_group_check=True,
            )
        slot = bank[:, sl * BL:(sl + 1) * BL]
        if t < T - 1:
            # s_{t+1} += tau_t @ W2
            nc.tensor.matmul(
                slot,
                lhsT=w2s[:],
                rhs=Tv[:, t, :],
                start=False,
                stop=True,
                skip_group_check=True,
            )
            # tau_{t+1} = tanh(s_{t+1} + b2)
            nc.scalar.activation(Tv[:, t + 1, :], slot, Tanh, bias=b2s[:])
        if sl == SLOTS - 1:
            # y = h + tau for this bank's t-range (off critical path, DVE)
            nc.vector.tensor_add(
                Yv[:, m * SLOTS:(m + 1) * SLOTS, :],
                Hv[:, m * SLOTS:(m + 1) * SLOTS, :],
                Tv[:, m * SLOTS:(m + 1) * SLOTS, :],
            )

    # ---- phase C: classifier out = y @ Wc + bc ----
    cls_psum = ctx.enter_context(
        tc.tile_pool(name=pfx + "cls", bufs=2, space="PSUM"))
    osb_pool = ctx.enter_context(tc.tile_pool(name=pfx + "osb", bufs=3))
    for b in range(BL):
        ps = cls_psum.tile([P, 4 * C], f32, tag="cls")
        for k in range(4):
            nc.tensor.matmul(
                ps[:, k * C:(k + 1) * C],
                lhsT=ybuf[:, b * T + k * P: b * T + (k + 1) * P],
                rhs=wcb[:],
                start=True,
                stop=True,
            )
        osb = osb_pool.tile([P, 4 * C], f32)
        nc.vector.tensor_add(osb[:], ps[:], bcb4[:])
        nc.sync.dma_start(
            outd[b].rearrange("(k p) c -> p k c", p=P),
            osb[:].rearrange("p (k c) -> p k c", c=C),
        )


def build_nc(nrep=1):
    nc = bacc.Bacc("TRN2", target_bir_lowering=False, debug=False,
                   num_devices=NCORES)
    x = nc.dram_tensor("inputs", [BL, T, D], f32, kind="ExternalInput").ap()
    w1 = nc.dram_tensor("W1", [D, U], f32, kind="ExternalInput").ap()
    b1 = nc.dram_tensor("b1", [U], f32, kind="ExternalInput").ap()
    w2 = nc.dram_tensor("W2", [U, U], f32, kind="ExternalInput").ap()
    b2 = nc.dram_tensor("b2", [U], f32, kind="ExternalInput").ap()
    wc = nc.dram_tensor("Wc", [U, C], f32, kind="ExternalInput").ap()
    bc = nc.dram_tensor("bc", [C], f32, kind="ExternalInput").ap()
    out = nc.dram_tensor("out", [BL, T, C], f32, kind="ExternalOutput").ap()

    with tile.TileContext(nc) as tc:
        import contextlib
        for rep in range(nrep):
            with contextlib.ExitStack() as ctx:
                build_body(nc, tc, ctx, x, w1, b1, w2, b2, wc, bc, out,
                           rep=rep)
    nc.finalize()
    return nc


def make_in_maps(inputs):
    xs = np.ascontiguousarray(np.asarray(inputs["inputs"], dtype=np.float32))
    shards = np.split(xs, NCORES, axis=0)
    common = {
        k: np.ascontiguousarray(np.asarray(inputs[k], dtype=np.float32))
        for k in ("W1", "b1", "W2", "b2", "Wc", "bc")
    }
    return [dict(inputs=shards[i], **common) for i in range(NCORES)]


def kernel(**inputs):
    nc = build_nc()
    in_maps = make_in_maps(inputs)
    res = bass_utils.run_bass_kernel_spmd(nc, in_maps, list(range(NCORES)))
    outs = [np.asarray(res.results[i]["out"]) for i in range(NCORES)]
    return np.concatenate(outs, axis=0).astype(np.float32)


# revision 12
# speedup vs baseline: 24.7588x; 2.1896x over previous
"""Trainium2 Bass kernel for the CustomRNN problem.

Math (per batch row):
    h_t   = tanh(x_t @ W1 + b1)                 (parallel over t)
    y_t   = h_t + tanh(y_{t-1} @ W2 + b2)       (serial scan over t)
    out_t = y_t @ Wc + bc                       (parallel over t)

Strategy (8 cores, data-parallel over batch; B_LOC = 32 rows/core):
  * Everything on-chip is kept in "transposed" layout [U, n].  The h/y
    buffers use b-major columns (n = b*T + t); tau uses t-major columns
    (n = t*32 + b) so the scan's ACT writes and z-matmul reads are
    contiguous.
  * Scan recurrence rewritten so the serial critical path is exactly
    PE -> ACT -> PE per step:
        g_t   = h_t @ W2            (parallel GEMM, accumulated directly
                                     into the scan PSUM banks)
        tau_t = tanh(s_t + b2)      (ACT, PSUM -> SBUF)
        s_{t+1} = g_t + tau_t @ W2  (PE matmul accumulate, start=False)
        y_t   = h_t + tau_t         (DVE, off the critical path)
  * x is transposed on-chip with PE transpose-mode matmuls (the DMA
    xbar path serializes ~1.3us/tile globally - too slow).
  * fp32 matmuls on trn2 lower to 2x(LDWEIGHTS+MATMUL) hi/lo passes;
    the scan matmul dtype is configurable (bf16 = 1 pass).
"""

import contextlib

import numpy as np

import concourse.bacc as bacc
import concourse.bass as bass
import concourse.mybir as mybir
import concourse.tile as tile
from concourse import bass_utils

B, T, D, U, C = 256, 512, 128, 128, 64
NCORES = 8
BL = B // NCORES  # 32 batch rows per core
P = 128
SLOTS = 16  # scan slots per PSUM bank
NBANKS = T // SLOTS  # 32

f32 = mybir.dt.float32
bf16 = mybir.dt.bfloat16
Tanh = mybir.ActivationFunctionType.Tanh

# dtype knobs
SCAN_DT = bf16   # dtype of tau and W2 for the serial z-matmul
H_DT = bf16      # dtype of h buffer (feeds g-matmul + y-add)


def build_body(nc, tc, ctx, x, w1d, b1d, w2d, b2d, wcd, bcd, outd, rep=0):
    pfx = f"r{rep}_"
    const = ctx.enter_context(tc.tile_pool(name=pfx + "const", bufs=1))
    big = ctx.enter_context(tc.tile_pool(name=pfx + "big", bufs=1))

    # ---- constants ----
    w1f = const.tile([D, U], f32)
    nc.sync.dma_start(w1f[:], w1d[:])
    w1s = const.tile([D, U], bf16)
    nc.vector.tensor_copy(w1s[:], w1f[:])
    w2f = const.tile([U, U], f32)
    nc.sync.dma_start(w2f[:], w2d[:])
    if SCAN_DT == f32:
        w2s = w2f
    else:
        w2s = const.tile([U, U], SCAN_DT)
        nc.vector.tensor_copy(w2s[:], w2f[:])
    if H_DT == f32:
        w2g = w2f
    else:
        w2g = const.tile([U, U], H_DT, name="w2g")
        nc.vector.tensor_copy(w2g[:], w2f[:])
    wcf = const.tile([U, C], f32)
    nc.sync.dma_start(wcf[:], wcd[:])
    wcb = const.tile([U, C], H_DT, name="wcb")
    nc.vector.tensor_copy(wcb[:], wcf[:])
    b1s = const.tile([U, 1], f32)
    nc.sync.dma_start(b1s[:], b1d.unsqueeze(1))
    b2s = const.tile([U, 1], f32)
    nc.sync.dma_start(b2s[:], b2d.unsqueeze(1))
    zero32 = const.tile([U, BL], f32)
    nc.vector.memset(zero32[:], 0.0)
    ones1 = const.tile([1, P], f32)
    nc.vector.memset(ones1[:], 1.0)
    bc1 = const.tile([1, C], f32)
    nc.sync.dma_start(bc1[:], bcd.unsqueeze(0))
    # identity for PE transpose
    idn = const.tile([P, P], bf16, name="idn")
    from concourse.masks import make_identity
    make_identity(nc, idn)

    # ---- big SBUF buffers ----
    # h, y: b-major columns (n = b*T + t); tau: t-major (n = t*BL + b)
    hbuf = big.tile([P, BL * T], H_DT)   # h_t (transposed layout)
    taub = big.tile([P, BL * T], SCAN_DT)
    ybuf = big.tile([P, BL * T], bf16)   # y = h + tau, classifier lhsT

    Hv = hbuf[:].rearrange("p (b t) -> p t b", b=BL, t=T)

    # ---- phase A: x load, cast, PE-transpose, input GEMM ----
    xa_pool = ctx.enter_context(tc.tile_pool(name=pfx + "xa", bufs=3))
    xb_pool = ctx.enter_context(tc.tile_pool(name=pfx + "xb", bufs=3))
    xt_pool = ctx.enter_context(tc.tile_pool(name=pfx + "xt", bufs=3))

    with tc.tile_pool(name=pfx + "ph", bufs=2, space="PSUM") as ph_psum, \
         tc.tile_pool(name=pfx + "tp", bufs=2, space="PSUM") as tp_psum:
        # bc broadcast tile via K=1 matmul (bcb4 = ones^T @ bc, tiled 4x)
        psmall = ph_psum.tile([P, C], f32, tag="ph")
        nc.tensor.matmul(psmall[:], lhsT=ones1[:], rhs=bc1[:], start=True,
                         stop=True)
        bcb4 = const.tile([P, 4 * C], f32)
        for k in range(4):
            nc.vector.tensor_copy(bcb4[:, k * C:(k + 1) * C], psmall[:])

        for b in range(BL):
            xa = xa_pool.tile([P, T], f32)
            # x[b] is [T, D]; rows t = a*128 + p onto partition p
            nc.sync.dma_start(xa[:], x[b].rearrange("(a p) d -> p a d", p=P))
            xb = xb_pool.tile([P, T], bf16)
            nc.vector.tensor_copy(xb[:], xa[:])
            xt = xt_pool.tile([P, T], bf16)
            for a in range(4):
                # PE transpose: [128(t'),128(d)] -> psum [128(d),128(t')]
                tp = tp_psum.tile([P, P], bf16, tag="tp")
                nc.tensor.transpose(tp[:], xb[:, a * P:(a + 1) * P], idn[:])
                nc.vector.tensor_copy(xt[:, a * P:(a + 1) * P], tp[:])
            ph = ph_psum.tile([P, T], f32, tag="ph")
            nc.tensor.matmul(ph[:], lhsT=w1s[:], rhs=xt[:], start=True,
                             stop=True)
            nc.scalar.activation(hbuf[:, b * T:(b + 1) * T], ph[:], Tanh,
                                 bias=b1s[:])

    # ---- phase B: the serial scan ----
    scan_psum = ctx.enter_context(
        tc.tile_pool(name=pfx + "scan", bufs=4, space="PSUM"))

    # tau_0 = tanh(0 + b2); tau is t-major: tau_t = taub[:, t*BL:(t+1)*BL]
    nc.scalar.activation(taub[:, 0:BL], zero32[:], Tanh, bias=b2s[:])

    bank = None
    for t in range(T):
        m, sl = divmod(t, SLOTS)
        if sl == 0:
            bank = scan_psum.tile([P, SLOTS * BL], f32, tag="bank")
            # g for this bank: slot sl' holds g_{16m+sl'} = h_{16m+sl'} @ W2
            nc.tensor.matmul(
                bank[:],
                lhsT=w2g[:],
                rhs=Hv[:, m * SLOTS:(m + 1) * SLOTS, :],
                start=True,
                stop=False,
                skip_group_check=True,
            )
        slot = bank[:, sl * BL:(sl + 1) * BL]
        if t < T - 1:
            # s_{t+1} += tau_t @ W2
            nc.tensor.matmul(
                slot,
                lhsT=w2s[:],
                rhs=taub[:, t * BL:(t + 1) * BL],
                start=False,
                stop=True,
                skip_group_check=True,
            )
            # tau_{t+1} = tanh(s_{t+1} + b2)
            nc.scalar.activation(taub[:, (t + 1) * BL:(t + 2) * BL], slot,
                                 Tanh, bias=b2s[:])

    # ---- phase B2: y = h + tau (tau is t-major -> strided read) ----
    Tv = taub[:].rearrange("p (t b) -> p t b", b=BL, t=T)
    Yv = ybuf[:].rearrange("p (b t) -> p t b", b=BL, t=T)
    for m in range(NBANKS):
        nc.vector.tensor_add(
            Yv[:, m * SLOTS:(m + 1) * SLOTS, :],
            Hv[:, m * SLOTS:(m + 1) * SLOTS, :],
            Tv[:, m * SLOTS:(m + 1) * SLOTS, :],
        )

    # ---- phase C: classifier out = y @ Wc + bc ----
    cls_psum = ctx.enter_context(
        tc.tile_pool(name=pfx + "cls", bufs=2, space="PSUM"))
    osb_pool = ctx.enter_context(tc.tile_pool(name=pfx + "osb", bufs=3))
    for b in range(BL):
        ps = cls_psum.tile([P, 4 * C], f32, tag="cls")
        for k in range(4):
            nc.tensor.matmul(
                ps[:, k * C:(k + 1) * C],
                lhsT=ybuf[:, b * T + k * P: b * T + (k + 1) * P],
                rhs=wcb[:],
                start=True,
                stop=True,
            )
        osb = osb_pool.tile([P, 4 * C], f32)
        nc.vector.tensor_add(osb[:], ps[:], bcb4[:])
        nc.sync.dma_start(
            outd[b].rearrange("(k p) c -> p k c", p=P),
            osb[:].rearrange("p (k c) -> p k c", c=C),
        )


def build_nc(nrep=1):
    nc = bacc.Bacc("TRN2", target_bir_lowering=False, debug=False,
                   num_devices=NCORES)
    x = nc.dram_tensor("inputs", [BL, T, D], f32, kind="ExternalInput").ap()
    w1 = nc.dram_tensor("W1", [D, U], f32, kind="ExternalInput").ap()
    b1 = nc.dram_tensor("b1", [U], f32, kind="ExternalInput").ap()
    w2 = nc.dram_tensor("W2", [U, U], f32, kind="ExternalInput").ap()
    b2 = nc.dram_tensor("b2", [U], f32, kind="ExternalInput").ap()
    wc = nc.dram_tensor("Wc", [U, C], f32, kind="ExternalInput").ap()
    bc = nc.dram_tensor("bc", [C], f32, kind="ExternalInput").ap()
    out = nc.dram_tensor("out", [BL, T, C], f32, kind="ExternalOutput").ap()

    with tile.TileContext(nc) as tc:
        for rep in range(nrep):
            with contextlib.ExitStack() as ctx:
                build_body(nc, tc, ctx, x, w1, b1, w2, b2, wc, bc, out,
                           rep=rep)
    nc.finalize()
    return nc


def make_in_maps(inputs):
    xs = np.ascontiguousarray(np.asarray(inputs["inputs"], dtype=np.float32))
    shards = np.split(xs, NCORES, axis=0)
    common = {
        k: np.ascontiguousarray(np.asarray(inputs[k], dtype=np.float32))
        for k in ("W1", "b1", "W2", "b2", "Wc", "bc")
    }
    return [dict(inputs=shards[i], **common) for i in range(NCORES)]


def kernel(**inputs):
    nc = build_nc()
    in_maps = make_in_maps(inputs)
    res = bass_utils.run_bass_kernel_spmd(nc, in_maps, list(range(NCORES)))
    outs = [np.asarray(res.results[i]["out"]) for i in range(NCORES)]
    return np.concatenate(outs, axis=0).astype(np.float32)
